# revision 3
# baseline (speedup 1.0000x reference)
"""Trainium2 Bass kernel for nn_CausalSelfAttention_6442450944521.

Sparse-attention causal self-attention block:
  B=4, T=2048 (rows<512: full attention over cols<512; rows>=512: causal),
  E=1024, H=16, D=64.

Sharding: batch (4) x head-group (2 groups of 8 heads) across 8 cores.
Each core computes, for its (batch b, head-group g):
  qkv^T projections (Q^T,K^T in pair layout [128,T]: head 2p in rows 0-63,
  head 2p+1 in rows 64-127; V in [T,D] layout), block-sparse attention via
  S^T = K Q^T, and its row-slice of the output projection. The two
  head-group partials per batch are summed on the host; v-bias and
  proj-bias are folded in exactly on the host.

Perf structure:
  - S^T matmuls are row-tiled on the PE: the two heads of a pair occupy
    disjoint 64-row halves of the 128x128 array (tile_position=(0,0) and
    (64,0)) and run concurrently, so the K=64 contraction no longer wastes
    half the array.
  - exp() runs on 1024-wide PSUM bins (2 k-tiles per ACTIVATE) to amortize
    the ~350-cycle per-op ACT overhead; diagonal (causal-frontier) blocks
    are packed tightly into two smaller bins with prebuilt combined masks.
  - q/k bias adds moved from ScalarE (the exp bottleneck) to VectorE
    tensor_scalar ops.
  - softmax denominators ride as a ones-column in the V stationary;
    normalization multiplies PSUM directly (no intermediate copy), with
    the reciprocal computed on a DMA-broadcast of the denominator row.
  - output partials are written bf16 (halves the output DMA).
"""

import os
import sys

if "/opt/trn_rl_repo" not in sys.path:
    sys.path.insert(0, "/opt/trn_rl_repo")

import numpy as np

# Problem constants (hardcoded per harness contract).
B = 4
T = 2048
E = 1024
H = 16
D = 64
NCORES = 8
HPC = H // 2          # heads per core = 8
ESL = HPC * D         # per-core E-slice = 512
P = 128               # SBUF/PSUM partitions
TG = 512              # matmul moving-dim tile (q-group width)
NTG = T // TG         # 4
NTT = T // P          # 16
NEC = E // P          # 8 contraction chunks over E
NPAIR = HPC // 2      # 4 head-pair tiles

_CACHE = {}


def _build_bins(qg):
    """Per q-group: list of (entries, used_cols, is_diag); entry =
    (kt, s0, off, n): k-tile kt covers q columns [s0, TG) of the group,
    its S^T block sits at pss/pT columns [off, off+n)."""
    if qg == 0:
        return [([(0, 0, 0, TG), (1, 0, TG, TG)], 2 * TG, False),
                ([(2, 0, 0, TG), (3, 0, TG, TG)], 2 * TG, False)]
    nf = 4 * qg
    m0 = 4 * qg
    bins = [([(kt, 0, 0, TG), (kt + 1, 0, TG, TG)], 2 * TG, False)
            for kt in range(0, nf, 2)]
    bins.append(([(m0, 0, 0, TG), (m0 + 1, P, TG, TG - P)],
                 2 * TG - P, True))                               # 896 used
    bins.append(([(m0 + 2, 2 * P, 0, TG - 2 * P),
                  (m0 + 3, 3 * P, TG - 2 * P, TG - 3 * P)],
                 2 * TG - 5 * P, True))                           # 384 used
    return bins


def _build_program():
    import concourse.bass as bass
    import concourse.tile as tile
    from concourse import bacc, mybir

    f32 = mybir.dt.float32
    bf16 = mybir.dt.bfloat16

    nc = bacc.Bacc("TRN2", target_bir_lowering=False, debug=False,
                   num_devices=NCORES)

    xT = nc.dram_tensor("xT", [E, T], bf16, kind="ExternalInput").ap()
    wq = nc.dram_tensor("wq", [E, ESL], bf16, kind="ExternalInput").ap()
    wk = nc.dram_tensor("wk", [E, ESL], bf16, kind="ExternalInput").ap()
    wv = nc.dram_tensor("wv", [E, ESL], bf16, kind="ExternalInput").ap()
    wp = nc.dram_tensor("wp", [ESL, E], bf16, kind="ExternalInput").ap()
    bq = nc.dram_tensor("bq", [ESL, 1], f32, kind="ExternalInput").ap()
    bk = nc.dram_tensor("bk", [ESL, 1], f32, kind="ExternalInput").ap()
    trimask = nc.dram_tensor("trimask", [P, TG], bf16,
                             kind="ExternalInput").ap()
    out = nc.dram_tensor("out", [T, E], bf16, kind="ExternalOutput").ap()

    with tile.TileContext(nc) as tc:
        _body(nc, tc, tile, mybir, bass,
              xT, wq, wk, wv, wp, bq, bk, trimask, out)

    nc.compile()
    return nc


def _body(nc, tc, tile, mybir, bass,
          xT, wq, wk, wv, wp, bq, bk, trimask, out):
    f32 = mybir.dt.float32
    bf16 = mybir.dt.bfloat16
    Exp = mybir.ActivationFunctionType.Exp
    from concourse.alu_op_type import AluOpType

    cms = {}

    def open_pool(name, bufs, space=None, side=None):
        kw = {}
        if space:
            kw["space"] = space
        if side:
            kw["side"] = side
        cm = tc.tile_pool(name=name, bufs=bufs, **kw)
        pool = cm.__enter__()
        cms[id(pool)] = cm
        return pool

    def close_pool(pool):
        cms.pop(id(pool)).__exit__(None, None, None)

    # ---- pools ----------------------------------------------------------
    singles = open_pool("singles", 1)
    yT_pool = open_pool("yTpool", 1)
    ps_qk = open_pool("psqk", 2, space="PSUM")      # [128,512] bufs=2
    ps_s = open_pool("pss", 2, space="PSUM")        # [128,1024] bufs=2
    ps_o = open_pool("pso", 2, space="PSUM")        # [128,512] bufs=2
    xr_pool = open_pool("xr", 1)                    # resident x^T (bf16)
    w_pool = open_pool("w", 1)                      # resident weights
    pT_pool = open_pool("pT", 6)
    rc_pool = open_pool("rc", 2)
    bc_pool = open_pool("bc", 2)
    ot_pool = open_pool("ot", 4)
    dr_pool = open_pool("dr", 2, space="DRAM")
    # right-stack: big attention-phase tensors
    qk_pool = open_pool("qkpool", 1, side="right")
    v_pool = open_pool("vpool", 1, side="right")

    # ---- resident tensors ------------------------------------------------
    # Combined diagonal masks: one DVE multiply per diag bin.
    mask896 = singles.tile([P, 2 * TG - P], bf16, tag="m896", name="m896")
    nc.sync.dma_start(out=mask896[:, 0:TG], in_=trimask[:, 0:TG])
    nc.sync.dma_start(out=mask896[:, TG:2 * TG - P], in_=trimask[:, 0:TG - P])
    mask384 = singles.tile([P, 3 * P], bf16, tag="m384", name="m384")
    nc.sync.dma_start(out=mask384[:, 0:2 * P], in_=trimask[:, 0:2 * P])
    nc.sync.dma_start(out=mask384[:, 2 * P:3 * P], in_=trimask[:, 0:P])
    bias_t = singles.tile([P, 2 * NPAIR], f32, tag="bias", name="bias")
    for pt in range(NPAIR):
        nc.sync.dma_start(out=bias_t[:, pt:pt + 1],
                          in_=bq[pt * P:(pt + 1) * P, :])
        nc.sync.dma_start(out=bias_t[:, NPAIR + pt:NPAIR + pt + 1],
                          in_=bk[pt * P:(pt + 1) * P, :])

    # x^T resident, loaded in two column halves (2KB/partition DMAs),
    # interleaved with Wv so the V phase can start after the first half.
    xr = []
    for ec in range(NEC):
        t = xr_pool.tile([P, T], bf16, tag=f"xr{ec}", name=f"xr{ec}")
        xr.append(t)
    wv_c = []
    for ec in range(NEC):
        t = w_pool.tile([P, ESL], bf16, tag="wv", name="wvc", bufs=NEC)
        wv_c.append(t)
    for ec in range(NEC):
        nc.sync.dma_start(out=xr[ec][:, 0:T // 2], in_=xT[ec * P:(ec + 1) * P,
                                                          0:T // 2])
        nc.sync.dma_start(out=wv_c[ec][:], in_=wv[ec * P:(ec + 1) * P, :])
    for ec in range(NEC):
        nc.sync.dma_start(out=xr[ec][:, T // 2:T],
                          in_=xT[ec * P:(ec + 1) * P, T // 2:T])

    yT_t = [yT_pool.tile([P, T], bf16, tag=f"yT{i}", name=f"yT{i}")
            for i in range(NPAIR)]
    qT_t = [qk_pool.tile([P, T], bf16, tag=f"qT{i}", name=f"qT{i}")
            for i in range(NPAIR)]
    # K^T pair layout [128, T]: head 2p in rows 0-63, head 2p+1 in 64-127.
    kT_t = [qk_pool.tile([P, T], bf16, tag=f"kT{i}", name=f"kT{i}")
            for i in range(NPAIR)]
    # V per T-tile: per head [V(64) | ones | zeros(63)] = 128-col stationary.
    v_t = [v_pool.tile([P, HPC, P], bf16, tag=f"v{i}", name=f"v{i}")
           for i in range(NTT)]

    wq_c, wk_c, wp_c = {}, {}, {}

    # ---- V = x @ Wv ------------------------------------------------------
    for tt in range(NTT):
        ts_ = slice(tt * P, (tt + 1) * P)
        psv = ps_qk.tile([P, HPC, D], f32, tag="qk", name="psv", bufs=2)
        for ec in range(NEC):
            nc.tensor.matmul(psv[:, :, :], lhsT=xr[ec][:, ts_],
                             rhs=wv_c[ec][:],
                             start=(ec == 0), stop=(ec == NEC - 1))
        nc.gpsimd.memset(v_t[tt][:, :, D + 1:], 0.0)
        nc.gpsimd.memset(v_t[tt][:, :, D:D + 1], 1.0)
        nc.vector.tensor_copy(v_t[tt][:, :, 0:D], psv[:, :, :])

    # ---- per pair: Q/K projections, then the pair's two heads ------------
    for pt in range(NPAIR):
        for ec in range(NEC):
            t = w_pool.tile([P, P], bf16, tag="wq", name="wqc", bufs=2 * NEC)
            nc.sync.dma_start(out=t[:], in_=wq[ec * P:(ec + 1) * P,
                                              pt * P:(pt + 1) * P])
            wq_c[(pt, ec)] = t
            t = w_pool.tile([P, P], bf16, tag="wk", name="wkc", bufs=2 * NEC)
            nc.sync.dma_start(out=t[:], in_=wk[ec * P:(ec + 1) * P,
                                              pt * P:(pt + 1) * P])
            wk_c[(pt, ec)] = t
        for tg in range(NTG):
            cs = slice(tg * TG, (tg + 1) * TG)
            psq = ps_qk.tile([P, TG], f32, tag="qk", name="psq", bufs=2)
            for ec in range(NEC):
                nc.tensor.matmul(psq[:], lhsT=wq_c[(pt, ec)][:],
                                 rhs=xr[ec][:, cs],
                                 start=(ec == 0), stop=(ec == NEC - 1))
            nc.vector.tensor_scalar(out=qT_t[pt][:, cs], in0=psq[:],
                                    scalar1=bias_t[:, pt:pt + 1],
                                    scalar2=None, op0=AluOpType.add)
            psk = ps_qk.tile([P, TG], f32, tag="qk", name="psk", bufs=2)
            for ec in range(NEC):
                nc.tensor.matmul(psk[:], lhsT=wk_c[(pt, ec)][:],
                                 rhs=xr[ec][:, cs],
                                 start=(ec == 0), stop=(ec == NEC - 1))
            nc.vector.tensor_scalar(out=kT_t[pt][:, cs], in0=psk[:],
                                    scalar1=bias_t[:, NPAIR + pt:NPAIR + pt + 1],
                                    scalar2=None, op0=AluOpType.add)

        # ---- attention for heads 2pt, 2pt+1 (row-tiled S) ----
        heads = (2 * pt, 2 * pt + 1)
        for qg in range(NTG):
            qb = qg * TG
            bins = _build_bins(qg)
            nent = sum(len(b[0]) for b in bins)
            po = {}
            for h in heads:
                po[h] = ps_o.tile([P, TG], f32, tag="po", name=f"po{h % 2}",
                                  bufs=2)
            ei = {h: 0 for h in heads}
            for entries, used, is_diag in bins:
                pss_, pT_ = {}, {}
                for h in heads:
                    pss_[h] = ps_s.tile([P, 2 * TG], f32, tag="pss",
                                        name=f"pss{h % 2}", bufs=2)
                # S^T: both heads concurrently in disjoint PE row halves.
                for kt, s0, off, n in entries:
                    ks = slice(kt * P, (kt + 1) * P)
                    for h in heads:
                        rb = (h % 2) * 64
                        nc.tensor.matmul(
                            pss_[h][:, off:off + n],
                            lhsT=kT_t[pt][rb:rb + 64, ks],
                            rhs=qT_t[pt][rb:rb + 64, qb + s0:qb + TG],
                            start=True, stop=True,
                            tile_position=(rb, 0))
                for h in heads:
                    pT_[h] = pT_pool.tile([P, 2 * TG], bf16, tag="pT",
                                          name="pT")
                    nc.scalar.activation(pT_[h][:, 0:used],
                                         pss_[h][:, 0:used], Exp, scale=0.125)
                    if is_diag:
                        m = mask896 if used == 2 * TG - P else mask384
                        nc.vector.tensor_mul(pT_[h][:, 0:used],
                                             pT_[h][:, 0:used], m[:, 0:used])
                for h in heads:
                    for kt, s0, off, n in entries:
                        nc.tensor.matmul(po[h][:, s0:TG],
                                         lhsT=v_t[kt][:, h, :],
                                         rhs=pT_[h][:, off:off + n],
                                         start=(ei[h] == 0),
                                         stop=(ei[h] == nent - 1))
                        ei[h] += 1
            # normalize: den row -> DRAM -> partition-broadcast; the
            # reciprocal runs at base partition 0 (reciprocal_approx_*
            # misbehaves off base 0); multiply reads PSUM directly.
            for h in heads:
                rb = (h % 2) * 64
                den_s = rc_pool.tile([1, TG], f32, tag="den_s", name="den_s")
                nc.vector.tensor_copy(den_s[:], po[h][D:D + 1, :])
                den_d = dr_pool.tile([1, TG], f32, tag="den_d", name="den_d")
                nc.sync.dma_start(out=den_d[:], in_=den_s[:])
                bcast_in = bass.AP(
                    tensor=den_d.tensor, offset=den_d.offset,
                    ap=[[0, D]] + [list(a) for a in den_d.ap[1:]])
                bc = bc_pool.tile([D, TG], f32, tag="bc", name="bc")
                nc.sync.dma_start(out=bc[:], in_=bcast_in)
                rcp = rc_pool.tile([D, TG], f32, tag="rcp", name="rcp")
                nc.vector.reciprocal_approx_fast(out=rcp[:], in_=bc[:])
                nc.vector.tensor_tensor(
                    out=yT_t[pt][rb:rb + 64, qb:qb + TG],
                    in0=po[h][0:D, :], in1=rcp[:], op=AluOpType.mult)

    # ---- proj: out = y @ Wp (row-parallel partial, bf16) -----------------
    for c in range(NPAIR):
        for ng in range(E // TG):
            t = w_pool.tile([P, TG], bf16, tag="wp", name="wpc",
                            bufs=2 * NPAIR)
            nc.sync.dma_start(out=t[:], in_=wp[c * P:(c + 1) * P,
                                              ng * TG:(ng + 1) * TG])
            wp_c[(c, ng)] = t
    for tt in range(NTT):
        ts_ = slice(tt * P, (tt + 1) * P)
        for ng in range(E // TG):
            pp = ps_qk.tile([P, TG], f32, tag="qk", name="pp", bufs=2)
            for c in range(NPAIR):
                nc.tensor.matmul(pp[:], lhsT=yT_t[c][:, ts_],
                                 rhs=wp_c[(c, ng)][:],
                                 start=(c == 0), stop=(c == NPAIR - 1))
            ot = ot_pool.tile([P, TG], bf16, tag="ot", name="ot")
            nc.vector.tensor_copy(ot[:], pp[:])
            nc.sync.dma_start(out=out[ts_, ng * TG:(ng + 1) * TG], in_=ot[:])

    close_pool(ot_pool)
    close_pool(v_pool)
    close_pool(qk_pool)
    close_pool(dr_pool)
    close_pool(bc_pool)
    close_pool(rc_pool)
    close_pool(pT_pool)
    close_pool(w_pool)
    close_pool(xr_pool)
    close_pool(ps_o)
    close_pool(ps_s)
    close_pool(ps_qk)
    close_pool(yT_pool)
    close_pool(singles)


def _get_program():
    if "nc" not in _CACHE:
        _CACHE["nc"] = _build_program()
    return _CACHE["nc"]


def make_in_maps(x, W_qkv, b_qkv, W_proj):
    """Per-core input dicts: core c -> (batch c%4, head-group c//4)."""
    import ml_dtypes
    x = np.asarray(x, np.float32)
    W_qkv = np.asarray(W_qkv, np.float32)
    b_qkv = np.asarray(b_qkv, np.float32)
    tri = (np.arange(TG)[None, :] >= np.arange(P)[:, None]).astype(np.float32)
    cvt = lambda a: np.ascontiguousarray(a).astype(ml_dtypes.bfloat16)
    in_maps = []
    for c in range(NCORES):
        b, g = c % B, c // B
        gs = slice(g * ESL, (g + 1) * ESL)
        in_maps.append({
            "xT": cvt(x[b].T),
            "wq": cvt(W_qkv[:, 0 * E:1 * E][:, gs]),
            "wk": cvt(W_qkv[:, 1 * E:2 * E][:, gs]),
            "wv": cvt(W_qkv[:, 2 * E:3 * E][:, gs]),
            "wp": cvt(np.asarray(W_proj, np.float32)[gs, :]),
            "bq": np.ascontiguousarray(b_qkv[0 * E:1 * E][gs, None]),
            "bk": np.ascontiguousarray(b_qkv[1 * E:2 * E][gs, None]),
            "trimask": cvt(tri),
        })
    return in_maps


def gather_output(results, b_qkv, b_proj, W_proj):
    """Sum the two row-parallel partials per batch; fold v/proj biases."""
    b_qkv = np.asarray(b_qkv, np.float64)
    W_proj = np.asarray(W_proj, np.float64)
    b_v = b_qkv[2 * E:3 * E]
    const = b_v @ W_proj + np.asarray(b_proj, np.float64)
    out = np.empty((B, T, E), np.float32)
    for b in range(B):
        out[b] = (results[b]["out"].astype(np.float64) +
                  results[b + B]["out"].astype(np.float64) +
                  const).astype(np.float32)
    return out


def run_on_hw(inputs, trace=False, **kwargs):
    from concourse.bass_utils import run_bass_kernel_spmd
    nc = _get_program()
    in_maps = make_in_maps(inputs["x"], inputs["W_qkv"], inputs["b_qkv"],
                           inputs["W_proj"])
    res = run_bass_kernel_spmd(nc, in_maps, list(range(NCORES)), trace=trace,
                               **kwargs)
    out = gather_output(res.results, inputs["b_qkv"], inputs["b_proj"],
                        inputs["W_proj"])
    return out, res


def kernel(x, W_qkv, b_qkv, W_proj, b_proj):
    out, _ = run_on_hw({"x": x, "W_qkv": W_qkv, "b_qkv": b_qkv,
                        "W_proj": W_proj, "b_proj": b_proj})
    return out


# revision 9
# speedup vs baseline: 1.3399x; 1.3399x over previous
"""Trainium2 Bass kernel for nn_CausalSelfAttention_6442450944521.

Sparse-attention causal self-attention block:
  B=4, T=2048 (rows<512: full attention over cols<512; rows>=512: causal),
  E=1024, H=16, D=64.

Sharding: batch (4) x head-group (2 groups of 8 heads) across 8 cores.
Each core computes, for its (batch b, head-group g):
  qkv^T projections (Q^T,K^T in pair layout [128,T]: head 2p in rows 0-63,
  head 2p+1 in rows 64-127; V in [T,D] layout), block-sparse attention via
  S^T = K Q^T, and its row-slice of the output projection. The two
  head-group partials per batch are summed on the host; v-bias and
  proj-bias are folded in exactly on the host.

Perf structure:
  - S^T matmuls are row-tiled on the PE: the two heads of a pair occupy
    disjoint 64-row halves of the 128x128 array (tile_position=(0,0) and
    (64,0)) and run concurrently, so the K=64 contraction no longer wastes
    half the array.
  - exp() runs on 1024-wide PSUM bins (2 k-tiles per ACTIVATE) to amortize
    the ~350-cycle per-op ACT overhead; diagonal (causal-frontier) blocks
    are packed tightly into two smaller bins with prebuilt combined masks.
  - q/k bias adds moved from ScalarE (the exp bottleneck) to VectorE
    tensor_scalar ops.
  - softmax denominators ride as a ones-column in the V stationary;
    normalization multiplies PSUM directly (no intermediate copy), with
    the reciprocal computed on a DMA-broadcast of the denominator row.
  - output partials are written bf16 (halves the output DMA).
"""

import os
import sys

if "/opt/trn_rl_repo" not in sys.path:
    sys.path.insert(0, "/opt/trn_rl_repo")

import numpy as np

# Problem constants (hardcoded per harness contract).
B = 4
T = 2048
E = 1024
H = 16
D = 64
NCORES = 8
HPC = H // 2          # heads per core = 8
ESL = HPC * D         # per-core E-slice = 512
P = 128               # SBUF/PSUM partitions
TG = 512              # matmul moving-dim tile (q-group width)
NTG = T // TG         # 4
NTT = T // P          # 16
NEC = E // P          # 8 contraction chunks over E
NPAIR = HPC // 2      # 4 head-pair tiles

_CACHE = {}


def _build_bins(qg):
    """Per q-group: list of (entries, used_cols, is_diag); entry =
    (kt, s0, off, n): k-tile kt covers q columns [s0, TG) of the group,
    its S^T block sits at pss/pT columns [off, off+n)."""
    if qg == 0:
        return [([(0, 0, 0, TG), (1, 0, TG, TG)], 2 * TG, False),
                ([(2, 0, 0, TG), (3, 0, TG, TG)], 2 * TG, False)]
    nf = 4 * qg
    m0 = 4 * qg
    bins = [([(kt, 0, 0, TG), (kt + 1, 0, TG, TG)], 2 * TG, False)
            for kt in range(0, nf, 2)]
    bins.append(([(m0, 0, 0, TG), (m0 + 1, P, TG, TG - P)],
                 2 * TG - P, True))                               # 896 used
    bins.append(([(m0 + 2, 2 * P, 0, TG - 2 * P),
                  (m0 + 3, 3 * P, TG - 2 * P, TG - 3 * P)],
                 2 * TG - 5 * P, True))                           # 384 used
    return bins


def _build_program():
    import concourse.bass as bass
    import concourse.tile as tile
    from concourse import bacc, mybir

    f32 = mybir.dt.float32
    bf16 = mybir.dt.bfloat16

    nc = bacc.Bacc("TRN2", target_bir_lowering=False, debug=False,
                   num_devices=NCORES)

    xT = nc.dram_tensor("xT", [E, T], bf16, kind="ExternalInput").ap()
    wq = nc.dram_tensor("wq", [E, ESL], bf16, kind="ExternalInput").ap()
    wk = nc.dram_tensor("wk", [E, ESL], bf16, kind="ExternalInput").ap()
    wv = nc.dram_tensor("wv", [E, ESL], bf16, kind="ExternalInput").ap()
    wp = nc.dram_tensor("wp", [ESL, E], bf16, kind="ExternalInput").ap()
    bq = nc.dram_tensor("bq", [ESL, 1], f32, kind="ExternalInput").ap()
    bk = nc.dram_tensor("bk", [ESL, 1], f32, kind="ExternalInput").ap()
    trimask = nc.dram_tensor("trimask", [P, TG], bf16,
                             kind="ExternalInput").ap()
    out = nc.dram_tensor("out", [T, E], bf16, kind="ExternalOutput").ap()

    with tile.TileContext(nc) as tc:
        _body(nc, tc, tile, mybir, bass,
              xT, wq, wk, wv, wp, bq, bk, trimask, out)

    nc.compile()
    return nc


def _body(nc, tc, tile, mybir, bass,
          xT, wq, wk, wv, wp, bq, bk, trimask, out):
    f32 = mybir.dt.float32
    bf16 = mybir.dt.bfloat16
    Exp = mybir.ActivationFunctionType.Exp
    from concourse.alu_op_type import AluOpType

    cms = {}

    def open_pool(name, bufs, space=None, side=None):
        kw = {}
        if space:
            kw["space"] = space
        if side:
            kw["side"] = side
        cm = tc.tile_pool(name=name, bufs=bufs, **kw)
        pool = cm.__enter__()
        cms[id(pool)] = cm
        return pool

    def close_pool(pool):
        cms.pop(id(pool)).__exit__(None, None, None)

    # ---- pools ----------------------------------------------------------
    singles = open_pool("singles", 1)
    yT_pool = open_pool("yTpool", 1)
    ps_qk = open_pool("psqk", 2, space="PSUM")      # [128,512] bufs=2
    ps_s = open_pool("pss", 2, space="PSUM")        # [128,1024] bufs=2
    ps_o = open_pool("pso", 2, space="PSUM")        # [128,512] bufs=2
    ob_pool = open_pool("ob", 4)
    xr_pool = open_pool("xr", 1)                    # resident x^T (bf16)
    w_pool = open_pool("w", 1)                      # resident weights
    pT_pool = open_pool("pT", 6)
    rc_pool = open_pool("rc", 2)
    bc_pool = open_pool("bc", 2)
    ot_pool = open_pool("ot", 4)
    dr_pool = open_pool("dr", 2, space="DRAM")
    # right-stack: big attention-phase tensors
    qk_pool = open_pool("qkpool", 1, side="right")
    v_pool = open_pool("vpool", 1, side="right")

    # ---- resident tensors ------------------------------------------------
    # Combined diagonal masks: one DVE multiply per diag bin.
    mask896 = singles.tile([P, 2 * TG - P], bf16, tag="m896", name="m896")
    nc.sync.dma_start(out=mask896[:, 0:TG], in_=trimask[:, 0:TG])
    nc.sync.dma_start(out=mask896[:, TG:2 * TG - P], in_=trimask[:, 0:TG - P])
    mask384 = singles.tile([P, 3 * P], bf16, tag="m384", name="m384")
    nc.sync.dma_start(out=mask384[:, 0:2 * P], in_=trimask[:, 0:2 * P])
    nc.sync.dma_start(out=mask384[:, 2 * P:3 * P], in_=trimask[:, 0:P])
    bias_t = singles.tile([P, 2 * NPAIR], f32, tag="bias", name="bias")
    for pt in range(NPAIR):
        nc.sync.dma_start(out=bias_t[:, pt:pt + 1],
                          in_=bq[pt * P:(pt + 1) * P, :])
        nc.sync.dma_start(out=bias_t[:, NPAIR + pt:NPAIR + pt + 1],
                          in_=bk[pt * P:(pt + 1) * P, :])

    # x^T resident, loaded in two column halves (2KB/partition DMAs),
    # interleaved with Wv so the V phase can start after the first half.
    xr = []
    for ec in range(NEC):
        t = xr_pool.tile([P, T], bf16, tag=f"xr{ec}", name=f"xr{ec}")
        xr.append(t)
    wv_c = []
    for ec in range(NEC):
        t = w_pool.tile([P, ESL], bf16, tag="wv", name="wvc", bufs=NEC)
        wv_c.append(t)
    for ec in range(NEC):
        nc.sync.dma_start(out=xr[ec][:, 0:T // 2], in_=xT[ec * P:(ec + 1) * P,
                                                          0:T // 2])
        nc.sync.dma_start(out=wv_c[ec][:], in_=wv[ec * P:(ec + 1) * P, :])
    for ec in range(NEC):
        nc.sync.dma_start(out=xr[ec][:, T // 2:T],
                          in_=xT[ec * P:(ec + 1) * P, T // 2:T])

    yT_t = [yT_pool.tile([P, T], bf16, tag=f"yT{i}", name=f"yT{i}")
            for i in range(NPAIR)]
    qT_t = [qk_pool.tile([P, T], bf16, tag=f"qT{i}", name=f"qT{i}")
            for i in range(NPAIR)]
    # K^T pair layout [128, T]: head 2p in rows 0-63, head 2p+1 in 64-127.
    kT_t = [qk_pool.tile([P, T], bf16, tag=f"kT{i}", name=f"kT{i}")
            for i in range(NPAIR)]
    # V per T-tile: per head [V(64) | ones | zeros(63)] = 128-col stationary.
    v_t = [v_pool.tile([P, HPC, P], bf16, tag=f"v{i}", name=f"v{i}")
           for i in range(NTT)]

    wq_c, wk_c, wp_c = {}, {}, {}

    # ---- V = x @ Wv ------------------------------------------------------
    for tt in range(NTT):
        ts_ = slice(tt * P, (tt + 1) * P)
        psv = ps_qk.tile([P, HPC, D], f32, tag="qk", name="psv", bufs=2)
        for ec in range(NEC):
            nc.tensor.matmul(psv[:, :, :], lhsT=xr[ec][:, ts_],
                             rhs=wv_c[ec][:],
                             start=(ec == 0), stop=(ec == NEC - 1))
        nc.gpsimd.memset(v_t[tt][:, :, D + 1:], 0.0)
        nc.gpsimd.memset(v_t[tt][:, :, D:D + 1], 1.0)
        nc.vector.tensor_copy(v_t[tt][:, :, 0:D], psv[:, :, :])

    # ---- per pair: Q/K projections, then the pair's two heads ------------
    for pt in range(NPAIR):
        if pt == NPAIR - 1:
            # prefetch proj weights under the last pair's attention
            for c in range(NPAIR):
                for ng in range(E // TG):
                    t = w_pool.tile([P, TG], bf16, tag="wp", name="wpc",
                                    bufs=2 * NPAIR)
                    nc.sync.dma_start(out=t[:], in_=wp[c * P:(c + 1) * P,
                                                      ng * TG:(ng + 1) * TG])
                    wp_c[(c, ng)] = t
        for ec in range(NEC):
            t = w_pool.tile([P, P], bf16, tag="wq", name="wqc", bufs=2 * NEC)
            nc.sync.dma_start(out=t[:], in_=wq[ec * P:(ec + 1) * P,
                                              pt * P:(pt + 1) * P])
            wq_c[(pt, ec)] = t
            t = w_pool.tile([P, P], bf16, tag="wk", name="wkc", bufs=2 * NEC)
            nc.sync.dma_start(out=t[:], in_=wk[ec * P:(ec + 1) * P,
                                              pt * P:(pt + 1) * P])
            wk_c[(pt, ec)] = t
        for tg in range(NTG):
            cs = slice(tg * TG, (tg + 1) * TG)
            psq = ps_qk.tile([P, TG], f32, tag="qk", name="psq", bufs=2)
            for ec in range(NEC):
                nc.tensor.matmul(psq[:], lhsT=wq_c[(pt, ec)][:],
                                 rhs=xr[ec][:, cs],
                                 start=(ec == 0), stop=(ec == NEC - 1))
            nc.vector.tensor_scalar(out=qT_t[pt][:, cs], in0=psq[:],
                                    scalar1=bias_t[:, pt:pt + 1],
                                    scalar2=None, op0=AluOpType.add)
            psk = ps_qk.tile([P, TG], f32, tag="qk", name="psk", bufs=2)
            for ec in range(NEC):
                nc.tensor.matmul(psk[:], lhsT=wk_c[(pt, ec)][:],
                                 rhs=xr[ec][:, cs],
                                 start=(ec == 0), stop=(ec == NEC - 1))
            nc.vector.tensor_scalar(out=kT_t[pt][:, cs], in0=psk[:],
                                    scalar1=bias_t[:, NPAIR + pt:NPAIR + pt + 1],
                                    scalar2=None, op0=AluOpType.add)

        # ---- attention for heads 2pt, 2pt+1 (row-tiled S) ----
        heads = (2 * pt, 2 * pt + 1)
        for qg in range(NTG):
            qb = qg * TG
            bins = _build_bins(qg)
            nent = sum(len(b[0]) for b in bins)
            po = {}
            for h in heads:
                po[h] = ps_o.tile([P, TG], f32, tag="po", name=f"po{h % 2}",
                                  bufs=2)
            ei = {h: 0 for h in heads}
            for entries, used, is_diag in bins:
                pss_, pT_ = {}, {}
                for h in heads:
                    pss_[h] = ps_s.tile([P, 2 * TG], f32, tag="pss",
                                        name=f"pss{h % 2}", bufs=2)
                # S^T: both heads concurrently in disjoint PE row halves.
                for kt, s0, off, n in entries:
                    ks = slice(kt * P, (kt + 1) * P)
                    for h in heads:
                        rb = (h % 2) * 64
                        nc.tensor.matmul(
                            pss_[h][:, off:off + n],
                            lhsT=kT_t[pt][rb:rb + 64, ks],
                            rhs=qT_t[pt][rb:rb + 64, qb + s0:qb + TG],
                            start=True, stop=True,
                            tile_position=(rb, 0))
                for h in heads:
                    pT_[h] = pT_pool.tile([P, 2 * TG], bf16, tag="pT",
                                          name="pT")
                    nc.scalar.activation(pT_[h][:, 0:used],
                                         pss_[h][:, 0:used], Exp, scale=0.125)
                    if is_diag:
                        m = mask896 if used == 2 * TG - P else mask384
                        nc.vector.tensor_mul(pT_[h][:, 0:used],
                                             pT_[h][:, 0:used], m[:, 0:used])
                for h in heads:
                    for kt, s0, off, n in entries:
                        nc.tensor.matmul(po[h][:, s0:TG],
                                         lhsT=v_t[kt][:, h, :],
                                         rhs=pT_[h][:, off:off + n],
                                         start=(ei[h] == 0),
                                         stop=(ei[h] == nent - 1))
                        ei[h] += 1
            # Evacuate O+den rows quickly (frees the po bank), then
            # normalize off-PSUM: den row -> DRAM -> partition-broadcast,
            # reciprocal at base partition 0 (reciprocal_approx_* misbehaves
            # off base 0), final multiply on the idle gpsimd engine.
            for h in heads:
                rb = (h % 2) * 64
                ob = ob_pool.tile([D + 1, TG], f32, tag="ob", name="ob")
                nc.vector.tensor_copy(ob[:], po[h][0:D + 1, :])
                den_d = dr_pool.tile([1, TG], f32, tag="den_d", name="den_d")
                nc.sync.dma_start(out=den_d[:], in_=ob[D:D + 1, :])
                bcast_in = bass.AP(
                    tensor=den_d.tensor, offset=den_d.offset,
                    ap=[[0, D]] + [list(a) for a in den_d.ap[1:]])
                bc = bc_pool.tile([D, TG], f32, tag="bc", name="bc")
                nc.sync.dma_start(out=bc[:], in_=bcast_in)
                rcp = rc_pool.tile([D, TG], f32, tag="rcp", name="rcp")
                nc.vector.reciprocal_approx_fast(out=rcp[:], in_=bc[:])
                nc.gpsimd.tensor_tensor(
                    out=yT_t[pt][rb:rb + 64, qb:qb + TG],
                    in0=ob[0:D, :], in1=rcp[:], op=AluOpType.mult)

    # ---- proj: out = y @ Wp (row-parallel partial, bf16) -----------------
    for tt in range(NTT):
        ts_ = slice(tt * P, (tt + 1) * P)
        for ng in range(E // TG):
            pp = ps_qk.tile([P, TG], f32, tag="qk", name="pp", bufs=2)
            for c in range(NPAIR):
                nc.tensor.matmul(pp[:], lhsT=yT_t[c][:, ts_],
                                 rhs=wp_c[(c, ng)][:],
                                 start=(c == 0), stop=(c == NPAIR - 1))
            ot = ot_pool.tile([P, TG], bf16, tag="ot", name="ot")
            nc.vector.tensor_copy(ot[:], pp[:])
            nc.sync.dma_start(out=out[ts_, ng * TG:(ng + 1) * TG], in_=ot[:])

    close_pool(v_pool)
    close_pool(qk_pool)
    close_pool(dr_pool)
    close_pool(ot_pool)
    close_pool(bc_pool)
    close_pool(rc_pool)
    close_pool(pT_pool)
    close_pool(w_pool)
    close_pool(xr_pool)
    close_pool(ob_pool)
    close_pool(ps_o)
    close_pool(ps_s)
    close_pool(ps_qk)
    close_pool(yT_pool)
    close_pool(singles)


def _get_program():
    if "nc" not in _CACHE:
        _CACHE["nc"] = _build_program()
    return _CACHE["nc"]


def make_in_maps(x, W_qkv, b_qkv, W_proj):
    """Per-core input dicts: core c -> (batch c%4, head-group c//4)."""
    import ml_dtypes
    x = np.asarray(x, np.float32)
    W_qkv = np.asarray(W_qkv, np.float32)
    b_qkv = np.asarray(b_qkv, np.float32)
    tri = (np.arange(TG)[None, :] >= np.arange(P)[:, None]).astype(np.float32)
    cvt = lambda a: np.ascontiguousarray(a).astype(ml_dtypes.bfloat16)
    in_maps = []
    for c in range(NCORES):
        b, g = c % B, c // B
        gs = slice(g * ESL, (g + 1) * ESL)
        in_maps.append({
            "xT": cvt(x[b].T),
            "wq": cvt(W_qkv[:, 0 * E:1 * E][:, gs]),
            "wk": cvt(W_qkv[:, 1 * E:2 * E][:, gs]),
            "wv": cvt(W_qkv[:, 2 * E:3 * E][:, gs]),
            "wp": cvt(np.asarray(W_proj, np.float32)[gs, :]),
            "bq": np.ascontiguousarray(b_qkv[0 * E:1 * E][gs, None]),
            "bk": np.ascontiguousarray(b_qkv[1 * E:2 * E][gs, None]),
            "trimask": cvt(tri),
        })
    return in_maps


def gather_output(results, b_qkv, b_proj, W_proj):
    """Sum the two row-parallel partials per batch; fold v/proj biases."""
    b_qkv = np.asarray(b_qkv, np.float64)
    W_proj = np.asarray(W_proj, np.float64)
    b_v = b_qkv[2 * E:3 * E]
    const = b_v @ W_proj + np.asarray(b_proj, np.float64)
    out = np.empty((B, T, E), np.float32)
    for b in range(B):
        out[b] = (results[b]["out"].astype(np.float64) +
                  results[b + B]["out"].astype(np.float64) +
                  const).astype(np.float32)
    return out


def run_on_hw(inputs, trace=False, **kwargs):
    from concourse.bass_utils import run_bass_kernel_spmd
    nc = _get_program()
    in_maps = make_in_maps(inputs["x"], inputs["W_qkv"], inputs["b_qkv"],
                           inputs["W_proj"])
    res = run_bass_kernel_spmd(nc, in_maps, list(range(NCORES)), trace=trace,
                               **kwargs)
    out = gather_output(res.results, inputs["b_qkv"], inputs["b_proj"],
                        inputs["W_proj"])
    return out, res


def kernel(x, W_qkv, b_qkv, W_proj, b_proj):
    out, _ = run_on_hw({"x": x, "W_qkv": W_qkv, "b_qkv": b_qkv,
                        "W_proj": W_proj, "b_proj": b_proj})
    return out
